# revision 7
# baseline (speedup 1.0000x reference)
"""ClusterGCN 2-layer kernel for 8 Trainium2 NeuronCores (Bass/Tile).

Strategy (graph/data parallel, per the sharding hint):
  - Target nodes sharded 8 ways (12500/core, padded to 12544 = 98*128 tiles).
  - Each core owns the edges whose target (col) is in its shard, grouped by
    128-node target tile, with self loops added as explicit edges. Edge
    counts are padded to be uniform across cores (single SPMD program).
  - Feature gathers use the MoE dma_gather primitive (int16 indices), so the
    gather source is split into 4 chunks by src%4; each supertile of ST
    target tiles issues one big gather per chunk. Blocks of 128 edge slots
    are target-tile-pure so each block feeds one one-hot segment-sum matmul.
  - Layer 1 accumulates agg.T = sum_e x[src_e] (x) onehot(col_e) in PSUM
    [feat, tgt], applies deg_inv via a rank-1 broadcast, then the out/root
    weights + bias + ReLU, leaving hT resident in SBUF.
  - z2 = relu(h) @ W2_out.T per shard, AllGathered (only collective).
  - Layer 2 gathers z2 rows (same edge structure), accumulates [tgt, 64],
    applies deg_inv / root / bias, writes the output shard.

All index tables / transposed weights / iota / identity are precomputed on
the host and passed as extra per-core inputs.
"""
import math
import numpy as np

P = 128
NCH = 4          # gather-source chunks (int16 index limit)
ST = 8           # target tiles per gather supertile


class Cfg:
    def __init__(self, n=100000, e=1600000, cores=8, c_in=128, c_hid=128,
                 c_out=64, st=ST):
        self.N, self.E, self.CORES = n, e, cores
        self.C_IN, self.C_HID, self.C_OUT = c_in, c_hid, c_out
        self.NS = n // cores                      # nodes per shard
        assert self.NS * cores == n
        assert self.NS % NCH == 0
        self.T = math.ceil(self.NS / P)           # target tiles per core
        self.NSP = self.T * P                     # padded shard size
        assert self.NSP % NCH == 0
        assert n % NCH == 0
        self.ST = st
        self.NST = math.ceil(self.T / st)         # supertiles


class Plan:
    """Static (cross-core-uniform) edge layout."""
    def __init__(self, cfg, K4):
        self.K4 = K4                              # [T][NCH] blocks
        self.Ktot = [sum(K4[t]) for t in range(cfg.T)]
        self.KSUM = sum(self.Ktot)
        self.toff = np.concatenate([[0], np.cumsum(self.Ktot)]).astype(int)
        # supertile structure
        self.sts = [list(range(s * cfg.ST, min((s + 1) * cfg.ST, cfg.T)))
                    for s in range(cfg.NST)]
        # per (st, ch): NI and block base per tile
        self.NI = [[sum(K4[t][ch] for t in tiles) * P for ch in range(NCH)]
                   for tiles in self.sts]
        self.base_blk = []
        for tiles in self.sts:
            bb = {}
            for ch in range(NCH):
                acc = 0
                for t in tiles:
                    bb[(t, ch)] = acc
                    acc += K4[t][ch]
            self.base_blk.append(bb)
        # idx tensor column offsets per (st, ch), in int16 columns (NI/16)
        w = []
        for s in range(len(self.sts)):
            for ch in range(NCH):
                w.append(self.NI[s][ch] // 16)
        self.woff = np.concatenate([[0], np.cumsum(w)]).astype(int)
        self.WTOT = int(self.woff[-1])

    def wslice(self, s, ch):
        i = s * NCH + ch
        return int(self.woff[i]), int(self.woff[i + 1])

    def subcalls(self, s, ch, max_blocks):
        """Split the (s, ch) block run into contiguous sub-call ranges of
        <= max_blocks blocks, each starting on a 4-block boundary (so the
        int16 idx slice stays 64B-aligned), as [(b0, b1), ...]."""
        tiles = self.sts[s]
        K = sum(self.K4[t][ch] for t in tiles)
        if K == 0:
            return []
        out, b = [], 0
        while b < K:
            sz = min(max_blocks, K - b)
            out.append((b, b + sz))
            b += sz
        return out


def wrap_idxs(flat):
    """[NI] int -> [128, NI//16] int16: j -> (j%16, j//16), replicated x8."""
    ni = flat.shape[0]
    w = flat.reshape(ni // 16, 16).T.astype(np.int16)
    return np.tile(w, (8, 1))


def preprocess(cfg, x, edge_index, W1_out, b1_out, W1_root, W2_out, b2_out,
               W2_root):
    N, NS, NSP, T = cfg.N, cfg.NS, cfg.NSP, cfg.T

    row = np.asarray(edge_index[0], dtype=np.int64)
    col = np.asarray(edge_index[1], dtype=np.int64)
    keep = row != col
    r = row[keep].astype(np.int64)
    c = col[keep].astype(np.int64)

    deg = np.bincount(c, minlength=N).astype(np.float32) + 1.0
    deg_inv = (1.0 / deg).astype(np.float32)

    selfs = np.arange(N, dtype=np.int64)
    r = np.concatenate([r, selfs])
    c = np.concatenate([c, selfs])

    shard = c // NS
    lt = c - shard * NS
    tt = lt // P
    cl = lt % P
    ch = r % NCH

    # per (core, tile, chunk) counts -> uniform K4
    flat_key = (shard * T + tt) * NCH + ch
    cnt = np.bincount(flat_key, minlength=cfg.CORES * T * NCH
                      ).reshape(cfg.CORES, T, NCH)
    K4 = [[int(math.ceil(cnt[:, t, q].max() / P)) for q in range(NCH)]
          for t in range(T)]
    plan = Plan(cfg, K4)

    order = np.lexsort((ch, tt, shard))
    r_s, tt_s, cl_s, ch_s = r[order], tt[order], cl[order], ch[order]
    key_s = (shard[order] * T + tt_s) * NCH + ch_s
    seg = np.searchsorted(key_s, np.arange(cfg.CORES * T * NCH + 1))

    iota = np.broadcast_to(np.arange(P, dtype=np.float32), (P, P)).copy()
    ident = np.eye(P, dtype=np.float32)
    ones = np.ones((1, P), np.float32)
    w1o = np.ascontiguousarray(np.asarray(W1_out, np.float32).T)
    w1r = np.ascontiguousarray(np.asarray(W1_root, np.float32).T)
    w2o = np.ascontiguousarray(np.asarray(W2_out, np.float32).T)
    w2r = np.ascontiguousarray(np.asarray(W2_root, np.float32).T)
    b1c = np.asarray(b1_out, np.float32).reshape(-1, 1)
    b2r = np.asarray(b2_out, np.float32).reshape(1, -1)
    xf = np.ascontiguousarray(np.asarray(x, np.float32))

    in_maps, emus = [], []
    for cc in range(cfg.CORES):
        # per-(t,ch) slot arrays: src global id (pad src=chunk row 0 i.e.
        # global ch), col (-1 pad)
        src_slots = np.zeros((T, max(plan.Ktot) if T else 0), object)
        colv = np.full((P, plan.KSUM), -1.0, np.float32)
        # per-slot global src, laid out [p, column] like colv
        srcg = np.zeros((P, plan.KSUM), np.int64)
        for t in range(T):
            coff = int(plan.toff[t])
            ccol = 0
            for q in range(NCH):
                i0 = seg[(cc * T + t) * NCH + q]
                i1 = seg[(cc * T + t) * NCH + q + 1]
                K = K4[t][q]
                L = i1 - i0
                s_pad = np.full(K * P, q, np.int64)    # pad: chunk row 0
                c_pad = np.full(K * P, -1.0, np.float32)
                s_pad[:L] = r_s[i0:i1]
                c_pad[:L] = cl_s[i0:i1].astype(np.float32)
                srcg[:, coff + ccol:coff + ccol + K] = \
                    s_pad.reshape(K, P).T
                colv[:, coff + ccol:coff + ccol + K] = \
                    c_pad.reshape(K, P).T
                ccol += K
        # build wrapped idx tensors per (st, ch) call
        zrow = (srcg // NS) * NSP + (srcg % NS)       # padded z2 row
        idx1w = np.zeros((P, plan.WTOT), np.int16)
        idx2w = np.zeros((P, plan.WTOT), np.int16)
        for s, tiles in enumerate(plan.sts):
            for q in range(NCH):
                w0, w1 = plan.wslice(s, q)
                if w1 == w0:
                    continue
                flat1 = np.zeros((w1 - w0) * 16, np.int64)
                flat2 = np.zeros((w1 - w0) * 16, np.int64)
                pos = 0
                for t in tiles:
                    coff = int(plan.toff[t])
                    cbase = sum(K4[t][qq] for qq in range(q))
                    K = K4[t][q]
                    for k in range(K):
                        col_i = coff + cbase + k
                        flat1[pos:pos + P] = srcg[:, col_i] // NCH
                        flat2[pos:pos + P] = zrow[:, col_i] // NCH
                        pos += P
                assert pos == (w1 - w0) * 16
                idx1w[:, w0:w1] = wrap_idxs(flat1)
                idx2w[:, w0:w1] = wrap_idxs(flat2)

        xloc = np.zeros((NSP, cfg.C_IN), np.float32)
        xloc[:NS] = xf[cc * NS:(cc + 1) * NS]
        dloc = np.ones(NSP, np.float32)
        dloc[:NS] = deg_inv[cc * NS:(cc + 1) * NS]
        dcol = np.ascontiguousarray(dloc.reshape(T, P).T)
        drow = dloc.reshape(1, NSP)

        in_maps.append({
            "xsrc": xf, "xloc": xloc,
            "idx1w": idx1w, "idx2w": idx2w, "colv": colv,
            "dcol": dcol, "drow": drow,
            "w1o": w1o, "w1r": w1r, "w2o": w2o, "w2r": w2r,
            "b1c": b1c, "b2r": b2r, "ones": ones, "iota": iota,
            "ident": ident,
        })
        emus.append({"srcg": srcg, "zrow": zrow})
    return in_maps, plan, emus


def build_program(cfg, plan):
    import concourse.bass as bass  # noqa: F401
    import concourse.bacc as bacc
    import concourse.mybir as mybir
    import concourse.tile as tile

    FP = mybir.dt.float32
    I16 = mybir.dt.int16
    AF = mybir.ActivationFunctionType
    OP = mybir.AluOpType
    N, NSP, T = cfg.N, cfg.NSP, cfg.T
    CI, CH_, CO = cfg.C_IN, cfg.C_HID, cfg.C_OUT
    K4, KSUM = plan.K4, plan.KSUM
    NI_max = max(max(r) for r in plan.NI)

    nc = bacc.Bacc("TRN2", target_bir_lowering=False, debug=False,
                   num_devices=cfg.CORES)

    xsrc = nc.dram_tensor("xsrc", [N, CI], FP, kind="ExternalInput")
    xloc = nc.dram_tensor("xloc", [NSP, CI], FP, kind="ExternalInput")
    idx1w = nc.dram_tensor("idx1w", [P, plan.WTOT], I16, kind="ExternalInput")
    idx2w = nc.dram_tensor("idx2w", [P, plan.WTOT], I16, kind="ExternalInput")
    colv = nc.dram_tensor("colv", [P, KSUM], FP, kind="ExternalInput")
    dcol = nc.dram_tensor("dcol", [P, T], FP, kind="ExternalInput")
    drow = nc.dram_tensor("drow", [1, NSP], FP, kind="ExternalInput")
    w1o = nc.dram_tensor("w1o", [CI, CH_], FP, kind="ExternalInput")
    w1r = nc.dram_tensor("w1r", [CI, CH_], FP, kind="ExternalInput")
    w2o = nc.dram_tensor("w2o", [CH_, CO], FP, kind="ExternalInput")
    w2r = nc.dram_tensor("w2r", [CH_, CO], FP, kind="ExternalInput")
    b1c = nc.dram_tensor("b1c", [CH_, 1], FP, kind="ExternalInput")
    b2r = nc.dram_tensor("b2r", [1, CO], FP, kind="ExternalInput")
    ones = nc.dram_tensor("ones", [1, P], FP, kind="ExternalInput")
    iota = nc.dram_tensor("iota", [P, P], FP, kind="ExternalInput")
    ident = nc.dram_tensor("ident", [P, P], FP, kind="ExternalInput")

    out = nc.dram_tensor("out", [NSP, CO], FP, kind="ExternalOutput")
    z2l = nc.dram_tensor("z2l", [NSP, CO], FP)
    z2f = nc.dram_tensor("z2f", [cfg.CORES * NSP, CO], FP, addr_space="Shared")

    with tile.TileContext(nc) as tc:
        with (
            tc.tile_pool(name="cst", bufs=1) as cst,
            tc.tile_pool(name="hp", bufs=1) as hp,
            tc.tile_pool(name="gp", bufs=2) as gp,
            tc.tile_pool(name="ip", bufs=3) as ip,
            tc.tile_pool(name="sp", bufs=3) as sp,
            tc.tile_pool(name="xp", bufs=3) as xp,
            tc.tile_pool(name="wk", bufs=3) as wk,
            tc.tile_pool(name="ps_scat", bufs=3, space="PSUM") as ps_scat,
            tc.tile_pool(name="ps_mm", bufs=2, space="PSUM") as ps_mm,
            tc.tile_pool(name="ps_aux", bufs=3, space="PSUM") as ps_aux,
        ):
            def load_const(t_dram, shape, dtype=FP):
                t_sb = cst.tile(shape, dtype, tag=t_dram.name)
                nc.sync.dma_start(out=t_sb[:], in_=t_dram[:, :])
                return t_sb

            colv_sb = load_const(colv, [P, KSUM])
            dcol_sb = load_const(dcol, [P, T])
            drow_sb = load_const(drow, [1, NSP])
            w1o_sb = load_const(w1o, [CI, CH_])
            w1r_sb = load_const(w1r, [CI, CH_])
            w2o_sb = load_const(w2o, [CH_, CO])
            w2r_sb = load_const(w2r, [CH_, CO])
            b1c_sb = load_const(b1c, [CH_, 1])
            b2r_sb = load_const(b2r, [1, CO])
            ones_sb = load_const(ones, [1, P])
            iota_sb = load_const(iota, [P, P])
            ident_sb = load_const(ident, [P, P])

            hT = hp.tile([P, NSP], FP)

            def build_onehot_tc(t, q):
                """One-hot blocks for (tile t, chunk q): [P, K4*P]."""
                K = K4[t][q]
                cbase = sum(K4[t][qq] for qq in range(q))
                o = int(plan.toff[t]) + cbase
                s = sp.tile([P, K * P], FP, tag="s")
                nc.vector.tensor_tensor(
                    out=s[:].rearrange("p (k j) -> p k j", k=K),
                    in0=iota_sb[:].unsqueeze(1).to_broadcast([P, K, P]),
                    in1=colv_sb[:, o:o + K].unsqueeze(2).to_broadcast(
                        [P, K, P]),
                    op=OP.is_equal,
                )
                return s

            def gather_supertile(idxw_dram, table_ap, elem, estep, s, q, tag,
                                 max_blocks):
                """Gather the (s, q) block run as sub-calls of <= max_blocks
                blocks (the dma_gather throughput sweet spot). Returns a list
                of (b0, b1, tile); block bb lives in tile at local bb-b0."""
                NI = plan.NI[s][q]
                if NI == 0:
                    return []
                w0, w1 = plan.wslice(s, q)
                it = ip.tile([P, w1 - w0], I16, tag=tag + "i")
                nc.sync.dma_start(out=it[:], in_=idxw_dram[:, w0:w1])
                parts = []
                for j, (b0, b1) in enumerate(plan.subcalls(s, q, max_blocks)):
                    nb = b1 - b0
                    g = gp.tile([P, max_blocks * elem], FP, tag=tag)
                    nc.gpsimd.dma_gather(
                        out_ap=g[:, 0:nb * elem].rearrange(
                            "p (k j) -> p k j", k=nb),
                        in_ap=table_ap,
                        idxs_ap=it[:, b0 * 8:b1 * 8],
                        num_idxs=nb * P,
                        num_idxs_reg=nb * P,
                        elem_size=elem,
                        elem_step=estep,
                        single_packet=False,
                    )
                    parts.append((b0, b1, g))
                return parts

            def part_slice(parts, bb, elem):
                for b0, b1, g in parts:
                    if b0 <= bb < b1:
                        return g[:, (bb - b0) * elem:(bb - b0 + 1) * elem]
                raise AssertionError(bb)

            # ---------------- layer 1 ----------------
            GSZ = 4                 # L1 target tiles per PSUM bank
            for s, tiles in enumerate(plan.sts):
                groups = [tiles[i:i + GSZ] for i in range(0, len(tiles), GSZ)]
                # (group_idx, region, t, q, k) sequence in program order per
                # group, to place start/stop on bank-first/last matmuls
                seqs = [[] for _ in groups]
                for q in range(NCH):
                    for gi, grp in enumerate(groups):
                        for r, t in enumerate(grp):
                            for k in range(K4[t][q]):
                                seqs[gi].append((q, r, t, k))
                psG = [ps_scat.tile([P, len(grp) * P], FP, tag="scat",
                                    name=f"psG_{s}_{gi}")
                       for gi, grp in enumerate(groups)]
                done = [0] * len(groups)
                for q in range(NCH):
                    parts = gather_supertile(idx1w, xsrc[q::NCH, :], CI,
                                             CI * NCH, s, q, "g1", 16)
                    for gi, grp in enumerate(groups):
                        for r, t in enumerate(grp):
                            K = K4[t][q]
                            if K == 0:
                                continue
                            sOH = build_onehot_tc(t, q)
                            bb = plan.base_blk[s][(t, q)]
                            for k in range(K):
                                nc.tensor.matmul(
                                    out=psG[gi][:, r * P:(r + 1) * P],
                                    lhsT=part_slice(parts, bb + k, CI),
                                    rhs=sOH[:, k * P:(k + 1) * P],
                                    start=(done[gi] == 0),
                                    stop=(done[gi] == len(seqs[gi]) - 1),
                                )
                                done[gi] += 1
                # epilogue per group then per tile
                for gi, grp in enumerate(groups):
                    W = len(grp) * P
                    t0 = grp[0]
                    db = ps_aux.tile([P, W], FP, tag="aux")
                    nc.tensor.matmul(
                        out=db[:], lhsT=ones_sb[:],
                        rhs=drow_sb[:, t0 * P:t0 * P + W],
                        start=True, stop=True)
                    db_sb = wk.tile([P, W], FP, tag="dbsb")
                    nc.scalar.activation(out=db_sb[:], in_=db[:],
                                         func=AF.Copy)
                    aggTn = wk.tile([P, W], FP, tag="aggTn")
                    nc.vector.tensor_tensor(out=aggTn[:], in0=psG[gi][:],
                                            in1=db_sb[:], op=OP.mult)
                    for r, t in enumerate(grp):
                        tb = slice(t * P, (t + 1) * P)
                        x_t = xp.tile([P, CI], FP, tag="x")
                        nc.sync.dma_start(out=x_t[:], in_=xloc[tb, :])
                        xT = ps_aux.tile([P, P], FP, tag="aux")
                        nc.tensor.transpose(out=xT[:], in_=x_t[:],
                                            identity=ident_sb[:])
                        xT_sb = wk.tile([P, P], FP, tag="xTsb")
                        nc.vector.tensor_copy(out=xT_sb[:], in_=xT[:])

                        o1 = ps_mm.tile([P, P], FP, tag="mm")
                        nc.tensor.matmul(out=o1[:], lhsT=w1o_sb[:],
                                         rhs=aggTn[:, r * P:(r + 1) * P],
                                         start=True, stop=False)
                        nc.tensor.matmul(out=o1[:], lhsT=w1r_sb[:],
                                         rhs=xT_sb[:],
                                         start=False, stop=True)
                        nc.scalar.activation(out=hT[:, tb], in_=o1[:],
                                             func=AF.Relu, bias=b1c_sb[:])

                        z2p = ps_aux.tile([P, CO], FP, tag="aux")
                        nc.tensor.matmul(out=z2p[:], lhsT=hT[:, tb],
                                         rhs=w2o_sb[:], start=True, stop=True)
                        z2sb = wk.tile([P, CO], FP, tag="z2sb")
                        nc.vector.tensor_copy(out=z2sb[:], in_=z2p[:])
                        nc.sync.dma_start(out=z2l[tb, :], in_=z2sb[:])

            # ---------------- allgather z2 ----------------
            nc.gpsimd.collective_compute(
                "AllGather", mybir.AluOpType.bypass,
                replica_groups=[list(range(cfg.CORES))],
                ins=[z2l.ap().opt()],
                outs=[z2f.ap().opt()],
            )

            # ---------------- layer 2 ----------------
            for s, tiles in enumerate(plan.sts):
                seq = []
                for q in range(NCH):
                    for t in tiles:
                        for k in range(K4[t][q]):
                            seq.append((q, t, k))
                psG = ps_scat.tile([P, len(tiles) * CO], FP, tag="scat")
                done = 0
                for q in range(NCH):
                    parts = gather_supertile(idx2w, z2f[q::NCH, :], CO,
                                             CO * NCH, s, q, "g2", 8)
                    for r, t in enumerate(tiles):
                        K = K4[t][q]
                        if K == 0:
                            continue
                        sOH = build_onehot_tc(t, q)
                        bb = plan.base_blk[s][(t, q)]
                        for k in range(K):
                            nc.tensor.matmul(
                                out=psG[:, r * CO:(r + 1) * CO],
                                lhsT=sOH[:, k * P:(k + 1) * P],
                                rhs=part_slice(parts, bb + k, CO),
                                start=(done == 0),
                                stop=(done == len(seq) - 1),
                            )
                            done += 1
                for r, t in enumerate(tiles):
                    tb = slice(t * P, (t + 1) * P)
                    agg2n = wk.tile([P, CO], FP, tag="agg2n")
                    nc.vector.tensor_scalar(
                        out=agg2n[:], in0=psG[:, r * CO:(r + 1) * CO],
                        scalar1=dcol_sb[:, t:t + 1], scalar2=None,
                        op0=OP.mult)

                    rb = ps_mm.tile([P, CO], FP, tag="mm")
                    nc.tensor.matmul(out=rb[:], lhsT=hT[:, tb], rhs=w2r_sb[:],
                                     start=True, stop=False)
                    nc.tensor.matmul(out=rb[:], lhsT=ones_sb[:], rhs=b2r_sb[:],
                                     start=False, stop=True)

                    osb = wk.tile([P, CO], FP, tag="osb")
                    nc.vector.tensor_tensor(out=osb[:], in0=agg2n[:],
                                            in1=rb[:], op=OP.add)
                    nc.sync.dma_start(out=out[tb, :], in_=osb[:])

    nc.compile()
    return nc


def kernel(x, edge_index, W1_out, b1_out, W1_root, W2_out, b2_out, W2_root):
    from concourse import bass2jax

    cfg = Cfg()
    in_maps, plan, _ = preprocess(
        cfg, x, edge_index, W1_out, b1_out, W1_root, W2_out, b2_out, W2_root)
    nc = build_program(cfg, plan)
    results = bass2jax.run_bass_via_pjrt(nc, in_maps, n_cores=cfg.CORES)
    outs = [results[cc]["out"][:cfg.NS] for cc in range(cfg.CORES)]
    return np.concatenate(outs, axis=0).astype(np.float32)

